# revision 19
# baseline (speedup 1.0000x reference)
"""NT-Xent (SimCLR) loss on 8 Trainium2 NeuronCores.

Math (validated against the reference formulation in f64):
  z = concat(z_i, z_j)                      [N=4096, D=512]
  zn = z / max(||z||, eps);  sim = zn@zn.T / T   (T=0.5, sim in [-2, 2])
  logits row i = sim row i minus the self-diagonal entry, so
    lse_i  = log(sum_{j!=i} exp(sim_ij - C)) + C      with fixed C (sim bounded)
    loss   = mean_i (lse_i - sim[i, partner(i)])
    rank_i = #{j != i : sim_ij > sim[i, partner(i)]}  (partner ties at 0)
    avg_rank = mean_i rank_i

Sharding: core r owns 512 rows of sim. Host pre-normalizes z, scales by
sqrt(1/T) (so the bf16 gram IS sim), transposes to [D, N] and permutes
columns per core to [partner-block | own-block | rest] so the partner /
self diagonals sit at fixed compile-time offsets (one NEFF for all
cores; row stats are column-permutation invariant). Each core matmuls
its row block [512, 4096] in 512-col PSUM chunks, fuses exp+row-sum on
ScalarE (accum_out) and greater-than+count on VectorE (accum_out), and
extracts the two diagonals with an identity-mask multiply-reduce.
Device emits per-row (S_full, self_diag, pos, count); the host applies
the exact self-exclusion corrections and the final log/mean in f64.
"""

import numpy as np
import ml_dtypes

import concourse.bacc as bacc
import concourse.mybir as mybir
import concourse.tile as tile
from concourse.bass_utils import run_bass_kernel_spmd
from concourse.masks import make_identity

B = 2048
D = 512
N = 2 * B
NCORES = 8
RPC = N // NCORES  # rows of sim per core = 512
KT = D // 128      # k tiles = 4
MT = RPC // 128    # m tiles per core = 4
CHUNK = 512        # columns per PSUM chunk (one bank of f32)
NCH = N // CHUNK   # n chunks = 8
# Logsumexp shift: sim is bounded in [-2, 2] (cos/T, T=0.5), so exp(sim)
# never overflows f32 and no shift is needed (C = 0 keeps the device op
# bias-free, avoiding an extra cross-engine wait on the ACT instruction).
SHIFT = 0.0

_f32 = mybir.dt.float32
_bf16 = mybir.dt.bfloat16

_NC_CACHE = {}


def _emit(tc):
    nc = tc.nc
    rhs_d = nc.dram_tensor("rhs", [KT, 128, N], _bf16, kind="ExternalInput")[:]
    out_d = nc.dram_tensor("out", [128, 4 * MT + 1], _f32, kind="ExternalOutput")[:]

    with (
        tc.tile_pool(name="singles", bufs=1) as singles,
        tc.tile_pool(name="psum", bufs=8, space="PSUM") as psum,
        tc.tile_pool(name="scratch", bufs=3) as scratch,
        tc.tile_pool(name="acc", bufs=4) as acc,
    ):
        ident = singles.tile([128, 128], _f32)
        make_identity(nc, ident)

        # Stage the full [D, N] bf16 operand in SBUF: 4 k-tiles of
        # [128, 4096] (8 KiB/partition each). Split DMAs column-wise so
        # the first matmuls can start before the tail columns land.
        rhs_sb = []
        for k in range(KT):
            t = singles.tile([128, N], _bf16, tag=f"rhs{k}")
            for q in range(4):
                nc.sync.dma_start(
                    out=t[:, q * 1024 : (q + 1) * 1024],
                    in_=rhs_d[k, :, q * 1024 : (q + 1) * 1024],
                )
            rhs_sb.append(t)

        # One spare column (16): written by the sync-absorber op below and
        # ignored by the host. TensorTensor ISA encodes only ONE sync wait,
        # so the diag-extract muls must depend solely on the PE matmul; this
        # live TS op makes VectorE observe the GpSimd-built identity first.
        outs = singles.tile([128, 4 * MT + 1], _f32)
        nc.vector.tensor_scalar_mul(outs[:, 4 * MT : 4 * MT + 1], ident[:, 0:1], 0.0)

        for t in range(MT):
            pos = acc.tile([128, 1], _f32, tag="pos")
            dself = acc.tile([128, 1], _f32, tag="dself")
            eacc = acc.tile([128, NCH], _f32, tag="eacc")
            cacc = acc.tile([128, NCH], _f32, tag="cacc")
            # lhsT = own-block columns (permuted cols 512..1023) of this
            # m-tile; the same SBUF tiles feed both matmul operands.
            lo = RPC + 128 * t
            chunk_ps = {}
            for g in range(2):  # chunk groups of 4: fewer PE weight reloads
                group = range(4 * g, 4 * g + 4)
                for c in group:
                    chunk_ps[c] = psum.tile([128, CHUNK], _f32, tag="ps", name="ps")
                for k in range(KT):
                    lhsT = rhs_sb[k][:, lo : lo + 128]
                    for c in group:
                        nc.tensor.matmul(
                            chunk_ps[c][:],
                            lhsT,
                            rhs_sb[k][:, CHUNK * c : CHUNK * (c + 1)],
                            start=(k == 0),
                            stop=(k == KT - 1),
                        )
                for c in group:
                    ps = chunk_ps[c]
                    if c in (0, 1):
                        # c==0: partner diagonal -> pos; c==1: self
                        # diagonal -> dself. Exact: identity mask leaves
                        # one nonzero per row, sum of zeros is exact.
                        dj = scratch.tile([128, 128], _f32, tag="diagjunk", bufs=8)
                        nc.vector.tensor_mul(
                            dj[:], ps[:, 128 * t : 128 * (t + 1)], ident[:]
                        )
                        nc.vector.reduce_sum(
                            out=(pos if c == 0 else dself)[:],
                            in_=dj[:],
                            axis=mybir.AxisListType.X,
                        )
                    ej = scratch.tile([128, CHUNK], _bf16, tag="ej")
                    nc.scalar.activation(
                        out=ej[:],
                        in_=ps[:],
                        func=mybir.ActivationFunctionType.Exp,
                        accum_out=eacc[:, c : c + 1],
                    )
                    cj = scratch.tile([128, CHUNK], _bf16, tag="cj")
                    nc.vector.tensor_scalar(
                        out=cj[:],
                        in0=ps[:],
                        scalar1=pos[:],
                        scalar2=None,
                        op0=mybir.AluOpType.is_gt,
                        op1=mybir.AluOpType.add,
                        accum_out=cacc[:, c : c + 1],
                    )
            nc.vector.reduce_sum(
                out=outs[:, 4 * t : 4 * t + 1], in_=eacc[:], axis=mybir.AxisListType.X
            )
            nc.vector.tensor_copy(out=outs[:, 4 * t + 1 : 4 * t + 2], in_=dself[:])
            nc.vector.tensor_copy(out=outs[:, 4 * t + 2 : 4 * t + 3], in_=pos[:])
            nc.vector.reduce_sum(
                out=outs[:, 4 * t + 3 : 4 * t + 4], in_=cacc[:], axis=mybir.AxisListType.X
            )

        nc.sync.dma_start(out=out_d, in_=outs[:])


def _build_nc():
    if "nc" in _NC_CACHE:
        return _NC_CACHE["nc"]
    # Bacc (not raw Bass): its compile() runs generate_event_semaphores,
    # which splits multi-sem waits into EventSemaphore instructions — the
    # hardware allows at most one sync wait per compute instruction.
    nc = bacc.Bacc("TRN2")
    with tile.TileContext(nc) as tc:
        _emit(tc)
    nc.compile()
    _NC_CACHE["nc"] = nc
    return nc


LAST_RESULT = None


def kernel(z_i, z_j, temperature=0.5):
    global LAST_RESULT
    z_i = np.asarray(z_i, dtype=np.float32)
    z_j = np.asarray(z_j, dtype=np.float32)
    assert z_i.shape == (B, D) and z_j.shape == (B, D)

    z = np.concatenate([z_i, z_j], axis=0)
    nrm = np.sqrt((z.astype(np.float64) ** 2).sum(axis=1, keepdims=True))
    nrm = np.maximum(nrm, 1e-8)
    zn = z / nrm
    # scale by sqrt(1/T) so the gram matrix equals sim = cos/T directly
    znb = (zn * np.sqrt(1.0 / float(temperature))).astype(ml_dtypes.bfloat16)
    znT = np.ascontiguousarray(znb.T)  # [D, N]

    rows = np.arange(N)
    in_maps = []
    for r in range(NCORES):
        own = rows[r * RPC : (r + 1) * RPC]
        part = (own + B) % N
        rest_mask = np.ones(N, dtype=bool)
        rest_mask[own] = False
        rest_mask[part] = False
        perm = np.concatenate([part, own, rows[rest_mask]])
        rhs = np.ascontiguousarray(znT[:, perm]).reshape(KT, 128, N)
        in_maps.append({"rhs": rhs})

    nc = _build_nc()
    res = run_bass_kernel_spmd(nc, in_maps, core_ids=list(range(NCORES)))
    LAST_RESULT = res

    tot_loss = 0.0
    tot_rank = 0.0
    for r in range(NCORES):
        o = np.asarray(res.results[r]["out"], dtype=np.float64)  # [128, 17]; col 16 unused
        for t in range(MT):
            S = o[:, 4 * t + 0]
            d = o[:, 4 * t + 1]
            p = o[:, 4 * t + 2]
            cnt = o[:, 4 * t + 3]
            Sc = S - np.exp(d - SHIFT)  # exclude the self term
            tot_loss += (np.log(Sc) + SHIFT - p).sum()
            tot_rank += (cnt - (d > p)).sum()

    loss = np.array(tot_loss / N, dtype=np.float32)
    avg_rank = np.array(tot_rank / N, dtype=np.float32)
    return loss, avg_rank
